# revision 1
# baseline (speedup 1.0000x reference)
"""Trainium2 Bass kernel for nn_RecommendationLoss.

Reference math (B=8192, L=1024, one positive label per row at a valid index):
  mask[b,l]  = l < len[b]
  bce_per[b] = sum_l mask*bce_el / (L * len)  where bce_el = -(lab*ln(s) + (1-lab)*ln(1-s))
  bce        = mean_b bce_per
  chosen[b]  = s[b, pos_b]
  hinge[b]   = sum_l neg_mask*relu(margin + s - chosen) / (len-1)   [valid iff len>=2]
  hinge      = sum_b hinge[b] / count(len>=2)
  sim        = -mean(similarity)
  out        = (hinge + bce + sim, hinge, bce, sim)

Device computes, per row (via per-128-row tiles, 8 tiles per core, 8 cores):
  chosen = sum_l labels*s                      (DVE tensor_tensor_reduce)
  sm     = (iota < len) * s                    (GpSimd scalar_tensor_tensor)
  A      = sum_l ln(1 - sm)                    (ACT Ln with accum_out; masked-out l give ln(1)=0)
  Eraw   = sum_l relu(sm + margin - chosen)    (DVE 2-op tensor_scalar with accum_out)
  E      = Eraw - (L - len)*relu(margin - chosen)   [tail correction, per-row scalars]
  bce row sum   = -(ln(chosen) + A - ln(1-chosen))
  hinge row val = (E - margin) * [len>=2]/(len-1)
Host does the trivial 1-D pieces (sim mean, valid count, final scalar combine) in f64.
"""

import sys

for _p in ("/opt/trn_rl_repo", "/opt/trn_rl_repo/concourse"):
    if _p not in sys.path:
        sys.path.insert(0, _p)

import numpy as np
import ml_dtypes

_bf16 = ml_dtypes.bfloat16

MARGIN = 0.1
B, L = 8192, 1024
N_CORES = 8
ROWS_PER_CORE = B // N_CORES      # 1024
P = 128                           # partitions
NT = ROWS_PER_CORE // P           # 8 tiles per core
# tiles whose E-reduce runs on DVE (max-identity) instead of ACT Relu,
# to balance the two pacing engines
DVE_E_TILES = frozenset({6})

_COMPILED = None


def _build():
    """Build + compile the per-core Bass program (same program on all cores)."""
    import concourse.bacc as bacc
    import concourse.tile as tile
    from concourse import mybir
    from concourse.alu_op_type import AluOpType as alu

    f32 = mybir.dt.float32
    bf16 = mybir.dt.bfloat16
    AF = mybir.ActivationFunctionType

    nc = bacc.Bacc("TRN2", target_bir_lowering=False, debug=False,
                   num_devices=N_CORES)

    scores = nc.dram_tensor("scores", [ROWS_PER_CORE, L], f32, kind="ExternalInput").ap()
    # labels are one-hot 0/1 — bf16 is a lossless encoding and halves DMA
    labels = nc.dram_tensor("labels", [ROWS_PER_CORE, L], bf16, kind="ExternalInput").ap()
    # per-row lengths as f32, laid out [P, NT]: column t = rows of tile t
    lens_d = nc.dram_tensor("lens", [P, NT], f32, kind="ExternalInput").ap()
    # stats out: columns [chosen | A | Eraw] x NT; final math runs on host
    out_d = nc.dram_tensor("out", [P, 3 * NT], f32, kind="ExternalOutput").ap()

    with tile.TileContext(nc) as tc:
        with (
            tc.tile_pool(name="const", bufs=1) as const,
            tc.tile_pool(name="io", bufs=5) as io,
            tc.tile_pool(name="work", bufs=3) as work,
            tc.tile_pool(name="stats", bufs=1) as stats,
        ):
            # allocation order unchanged (SBUF layout is perf-sensitive);
            # only DMA issue order moves: tile-0 data first, tiny lens after
            lens_sb = const.tile([P, NT], f32)
            iota = const.tile([P, L], f32)
            nc.gpsimd.iota(iota, pattern=[[1, L]], base=0, channel_multiplier=0,
                           allow_small_or_imprecise_dtypes=True)

            stats_sb = stats.tile([P, 3 * NT], f32)
            # 4-byte warmup DMA on ACT's idle queue: absorbs first-DMA
            # spin-up latency in parallel with the sync queue's real loads
            nc.scalar.dma_start(out=stats_sb[0:1, 0:1], in_=lens_d[0:1, 0:1])
            chosen_all = stats_sb[:, 0 * NT:1 * NT]
            A_all = stats_sb[:, 1 * NT:2 * NT]
            Eraw_all = stats_sb[:, 2 * NT:3 * NT]
            mc_all = stats.tile([P, NT], f32)      # margin - chosen (bias for Relu)

            for t in range(NT):
                rows = slice(t * P, (t + 1) * P)
                s_t = io.tile([P, L], f32)
                nc.sync.dma_start(out=s_t, in_=scores[rows, :])
                lab_t = io.tile([P, L], bf16)
                nc.sync.dma_start(out=lab_t, in_=labels[rows, :])
                if t == 0:
                    nc.sync.dma_start(out=lens_sb, in_=lens_d)

                # allocation order (junk, sm) is kept — SBUF layout is
                # perf-sensitive — but sm's op is EMITTED first: it only
                # needs s_t (ready before lab_t) and alone unblocks the Ln
                junk = work.tile([P, L], f32)
                sm = work.tile([P, L], f32)
                # sm = (iota < len) * s   [DVE scalar_tensor_tensor]
                nc.vector.scalar_tensor_tensor(
                    out=sm, in0=iota, scalar=lens_sb[:, t:t + 1], in1=s_t,
                    op0=alu.is_lt, op1=alu.mult)
                # chosen = sum_l labels * s
                nc.vector.scalar_tensor_tensor(
                    out=junk, in0=lab_t, scalar=0.0, in1=s_t,
                    op0=alu.bypass, op1=alu.mult,
                    accum_out=chosen_all[:, t:t + 1])
                # A = sum_l ln(1 - sm)   [ACT, fused accumulate]
                l1m = work.tile([P, L], f32)
                nc.scalar.activation(
                    out=l1m, in_=sm, func=AF.Ln, scale=-1.0, bias=1.0,
                    accum_out=A_all[:, t:t + 1])
                if t in DVE_E_TILES:
                    # Emax = sum_l max(sm, chosen - margin)  [DVE 2x tensor_scalar]
                    # host reconstructs Eraw = Emax + L*(margin - chosen)
                    cm_t = stats.tile([P, 1], f32, tag="cm")
                    nc.vector.tensor_scalar(
                        out=cm_t, in0=chosen_all[:, t:t + 1],
                        scalar1=-MARGIN, scalar2=None, op0=alu.add)
                    q_t = work.tile([P, L], f32, tag="q")
                    nc.vector.tensor_scalar(
                        out=q_t, in0=sm, scalar1=cm_t, scalar2=0.0,
                        op0=alu.max, op1=alu.add,
                        accum_out=Eraw_all[:, t:t + 1])
                else:
                    # mc = margin - chosen
                    nc.vector.tensor_scalar(
                        out=mc_all[:, t:t + 1], in0=chosen_all[:, t:t + 1],
                        scalar1=-1.0, scalar2=MARGIN, op0=alu.mult, op1=alu.add)
                    # Eraw = sum_l relu(sm + mc)  [ACT, per-partition bias, accum]
                    r_t = work.tile([P, L], f32)
                    nc.scalar.activation(
                        out=r_t, in_=sm, func=AF.Relu, bias=mc_all[:, t:t + 1],
                        scale=1.0, accum_out=Eraw_all[:, t:t + 1])

            # per-plane exports, each issued from the engine that produces the
            # plane's last value: chosen (DVE accum -> sync queue), A and Eraw
            # from ACT's own HWDGE right as its last Ln/Relu retire
            nc.sync.dma_start(out=out_d[:, 0:NT], in_=stats_sb[:, 0:NT])
            nc.scalar.dma_start(out=out_d[:, NT:2 * NT],
                                in_=stats_sb[:, NT:2 * NT])
            nc.scalar.dma_start(out=out_d[:, 2 * NT:3 * NT],
                                in_=stats_sb[:, 2 * NT:3 * NT])

    nc.compile()
    return nc


def _get_compiled():
    global _COMPILED
    if _COMPILED is None:
        _COMPILED = _build()
    return _COMPILED


def _make_in_maps(scores, labels, lens_f64):
    in_maps = []
    for c in range(N_CORES):
        rows = slice(c * ROWS_PER_CORE, (c + 1) * ROWS_PER_CORE)
        lv = lens_f64[rows].reshape(NT, P).T          # [P, NT], col t = tile t rows
        lab = np.ascontiguousarray(labels[rows])
        if lab.dtype == np.float32:
            # values are exactly 0.0/1.0 -> bf16 truncation is exact; the
            # bit-shift view is much faster than ml_dtypes astype
            lab = (lab.view(np.uint32) >> 16).astype(np.uint16).view(_bf16)
        else:
            lab = lab.astype(_bf16)
        in_maps.append({
            "scores": np.ascontiguousarray(scores[rows], dtype=np.float32),
            "labels": lab,
            "lens": np.ascontiguousarray(lv, dtype=np.float32),
        })
    return in_maps


def _combine(core_outs, lens_f64, sim_f64):
    """Host-side finals from per-core [P, 3*NT] stats tiles (f64 math)."""
    bsum = 0.0
    hsum = 0.0
    for c in range(N_CORES):
        o = np.asarray(core_outs[c], dtype=np.float64)
        chosen = o[:, 0 * NT:1 * NT]
        A = o[:, 1 * NT:2 * NT]
        Eraw = o[:, 2 * NT:3 * NT].copy()
        rows = slice(c * ROWS_PER_CORE, (c + 1) * ROWS_PER_CORE)
        lv = lens_f64[rows].reshape(NT, P).T          # [P, NT]
        mc = MARGIN - chosen
        for t in DVE_E_TILES:                         # device stored Emax there
            Eraw[:, t] += float(L) * mc[:, t]
        E = Eraw - (float(L) - lv) * np.maximum(mc, 0.0)
        bce_rows = -(np.log(chosen) + A - np.log1p(-chosen)) / (float(L) * lv)
        hv = np.where(lv >= 2.0, 1.0 / np.maximum(lv - 1.0, 1.0), 0.0)
        hinge_rows = (E - MARGIN) * hv
        bsum += bce_rows.sum()
        hsum += hinge_rows.sum()

    vcnt = float(np.count_nonzero(lens_f64 >= 2.0))
    bce = bsum / float(B)
    hinge = hsum / vcnt if vcnt > 0 else 0.0
    sim_loss = -sim_f64.mean()
    combined = hinge + bce + sim_loss
    return np.array([combined, hinge, bce, sim_loss], dtype=np.float32)


LAST_RESULTS = None  # BassKernelResults of the most recent run (for profiling)


def kernel(scores, candidate_lengths, labels, similarity_top_cand,
           _trace=False, _trace_kwargs=None):
    from concourse.bass_utils import run_bass_kernel_spmd

    global LAST_RESULTS
    nc = _get_compiled()

    scores = np.asarray(scores)
    labels = np.asarray(labels)
    lens_f64 = np.asarray(candidate_lengths).astype(np.float64)
    sim = np.asarray(similarity_top_cand).astype(np.float64)

    in_maps = _make_in_maps(scores, labels, lens_f64)
    res = run_bass_kernel_spmd(
        nc, in_maps, core_ids=list(range(N_CORES)),
        trace=_trace, **(_trace_kwargs or {}))
    LAST_RESULTS = res

    return _combine([res.results[c]["out"] for c in range(N_CORES)],
                    lens_f64, sim)

